# revision 21
# baseline (speedup 1.0000x reference)
"""Multi-head causal attention (B=2, T=2048, D=1024, H=16) on 8 trn2 cores.

Sharding: core c = (batch b, head-group g) with b = c//4, g = c%4.
Each core computes Q/K/V projections for its 4 heads (256 features),
causal attention, and its partial output projection; the host sums the
4 per-batch partials (the w_o all-reduce) and stacks batches.

All device matmuls run as float32r (full PE rate, near-fp32 accuracy).
Host pre-transposes x and weight slices so every matmul operand is
contraction-major; 1/sqrt(dh) is folded into wq on the host.
"""

import math

import ml_dtypes
import numpy as np

BF16NP = ml_dtypes.bfloat16

import concourse.bass as bass
from concourse import bacc
import concourse.mybir as mybir
import concourse.tile as tile
from concourse.bass_utils import run_bass_kernel_spmd

F32 = mybir.dt.float32
F32R = mybir.dt.float32r
AF = mybir.ActivationFunctionType
ALU = mybir.AluOpType
BF16 = mybir.dt.bfloat16

B, T, D, H = 2, 2048, 1024, 16
NCORES = 8
G = 4             # head groups (tensor parallel); cores = B * G
HPG = H // G      # 4 heads per core
DH = D // H       # 64 head dim
E = D // G        # 256 features per core
EB = E // 128     # 2 e-blocks of 128
KD = D // 128     # 8 contraction chunks for projections
TN = T // 512     # 4 512-wide t stripes
TC = T // 128     # 16 128-wide k/t chunks
HT = T // 2       # 1024-wide attention q-half


def build_nc():
    nc = bacc.Bacc(None)
    xqT = nc.declare_dram_parameter("xqT", [D, T], BF16, isOutput=False)
    xkT = nc.declare_dram_parameter("xkT", [D, T], BF16, isOutput=False)
    xvT = nc.declare_dram_parameter("xvT", [D, T], BF16, isOutput=False)
    wqT = nc.declare_dram_parameter("wqT", [D, E], BF16, isOutput=False)
    wkT = nc.declare_dram_parameter("wkT", [D, E], BF16, isOutput=False)
    wvT = nc.declare_dram_parameter("wvT", [D, E], BF16, isOutput=False)
    woT = nc.declare_dram_parameter("woT", [E, D], BF16, isOutput=False)
    outp = nc.declare_dram_parameter("outp", [T, D], F32, isOutput=True)

    with tile.TileContext(nc) as tc:
        with (
            tc.tile_pool(name="persist", bufs=1) as persist,
            tc.tile_pool(name="xs", bufs=16) as xs,
            tc.tile_pool(name="pt", bufs=3) as ptp,
            tc.tile_pool(name="ptd", bufs=2) as ptdp,
            tc.tile_pool(name="rsb", bufs=2) as rsbp,
            tc.tile_pool(name="odd", bufs=2) as oddp,
            tc.tile_pool(name="outs", bufs=4) as outsp,
            tc.tile_pool(name="ps", bufs=2, space="PSUM") as psp,
            tc.tile_pool(name="po", bufs=2, space="PSUM") as pop,
        ):
            wq_sb = persist.tile([128, KD, E], BF16, tag="wq")
            wk_sb = persist.tile([128, KD, E], BF16, tag="wk")
            wv_sb = persist.tile([128, KD, E], BF16, tag="wv")
            wo_sb = persist.tile([128, EB, D], BF16, tag="wo")
            QT = persist.tile([128, EB, T], BF16, tag="QT")
            KT = persist.tile([128, EB, T], BF16, tag="KT")
            Vp = persist.tile([128, TC, HPG, DH + 1], BF16, tag="Vp")
            ONpk = persist.tile([128, EB, T], BF16, tag="ONpk")

            nc.sync.dma_start(wq_sb[:], wqT[:, :].rearrange("(c p) e -> p c e", p=128))
            nc.sync.dma_start(wk_sb[:], wkT[:, :].rearrange("(c p) e -> p c e", p=128))
            nc.sync.dma_start(wv_sb[:], wvT[:, :].rearrange("(c p) e -> p c e", p=128))
            nc.sync.dma_start(wo_sb[:], woT[:, :].rearrange("(c p) d -> p c d", p=128))

            # ones column of Vp: P @ [V | 1] makes the softmax denominator
            # fall out of the PV matmul as psum row 64
            onesb = persist.tile([128, 512], BF16, tag="onesb")
            nc.vector.memset(onesb[:], 1.0)
            nc.vector.tensor_copy(
                Vp[:, :, :, DH : DH + 1],
                onesb[:, None, None, 0:1].broadcast_to([128, TC, HPG, 1]),
            )
            # four causal masks (one per kc%4): keep qq >= kq + 128*m
            msk = persist.tile([128, 4, 512], BF16, tag="msk")
            for m in range(4):
                nc.gpsimd.affine_select(
                    out=msk[:, m, :],
                    in_=onesb[:, :],
                    pattern=[[1, 512]],
                    compare_op=ALU.is_ge,
                    fill=0.0,
                    base=-(128 * m),
                    channel_multiplier=-1,
                )

            def load_x_stripe(xdram, n):
                tiles = []
                xr = xdram[:, :].rearrange("(c p) t -> p c t", p=128)
                for kd in range(KD):
                    t = xs.tile([128, 512], BF16, tag="x")
                    nc.sync.dma_start(t[:], xr[:, kd, 512 * n : 512 * n + 512])
                    tiles.append(t)
                return tiles

            # ---- Q/K projections: dest[e, t] = w[d, e].T @ x[d, t] ----
            for xdram, wsb, dest in ((xqT, wq_sb, QT), (xkT, wk_sb, KT)):
                for n in range(TN):
                    xc = load_x_stripe(xdram, n)
                    for eb in range(EB):
                        acc = psp.tile([128, 512], F32, tag="ps")
                        for kd in range(KD):
                            nc.tensor.matmul(
                                acc[:],
                                wsb[:, kd, 128 * eb : 128 * eb + 128],
                                xc[kd][:],
                                start=(kd == 0),
                                stop=(kd == KD - 1),
                            )
                        nc.vector.tensor_copy(
                            dest[:, eb, 512 * n : 512 * n + 512], acc[:]
                        )

            # ---- V projection, natural layout: V[t, e] = x[d, t].T @ w[d, e] ----
            for n in range(TN):
                xc = load_x_stripe(xvT, n)
                for sub in range(4):
                    tcc = 4 * n + sub
                    acc = psp.tile([128, E], F32, tag="ps")
                    for kd in range(KD):
                        nc.tensor.matmul(
                            acc[:],
                            xc[kd][:, 128 * sub : 128 * sub + 128],
                            wv_sb[:, kd, :],
                            start=(kd == 0),
                            stop=(kd == KD - 1),
                        )
                    nc.vector.tensor_copy(
                        Vp[:, tcc, :, 0:DH],
                        acc[:].rearrange("p (h d) -> p h d", h=HPG),
                    )

            # ---- attention per (head, q-half); pO double-buffers so the
            # next half's PV accumulation overlaps this half's softmax
            # normalization ----
            for h in range(HPG):
                eb, r0 = h // 2, 64 * (h % 2)
                for half in range(2):
                    q0 = HT * half
                    pO = pop.tile([128, HT], F32, tag="po")
                    # rows 65-95 feed stream_shuffle; only row 64 is real
                    nc.vector.memset(pO[64:96, :], 0.0)
                    kc_hi = 8 * (half + 1)
                    for kc in range(kc_hi):
                        jlo = max(kc // 4, 2 * half)
                        pS = psp.tile([128, HT], F32, tag="ps")
                        pe_t = ptp.tile([128, HT], BF16, tag="pt")
                        for jg in range(jlo, 2 * half + 2):
                            o = 512 * jg - q0
                            nc.tensor.matmul(
                                pS[:, o : o + 512],
                                KT[r0 : r0 + 64, eb, 128 * kc : 128 * kc + 128],
                                QT[r0 : r0 + 64, eb, 512 * jg : 512 * jg + 512],
                                start=True,
                                stop=True,
                            )
                        vo = 512 * jlo - q0
                        nc.scalar.activation(pe_t[:, vo:], pS[:, vo:], AF.Exp)
                        for jg in range(jlo, 2 * half + 2):
                            o = 512 * jg - q0
                            if jg == kc // 4:
                                # causal mask on the diagonal stripe
                                ptd = ptdp.tile([128, 512], BF16, tag="ptd")
                                nc.vector.tensor_tensor(
                                    out=ptd[:],
                                    in0=pe_t[:, o : o + 512],
                                    in1=msk[:, kc % 4, :],
                                    op=ALU.mult,
                                )
                                src = ptd[:]
                            else:
                                src = pe_t[:, o : o + 512]
                            nc.tensor.matmul(
                                pO[0:65, o : o + 512],
                                Vp[:, kc, h, :],
                                src,
                                start=(kc == 0),
                                stop=(kc == 4 * jg + 3),
                            )
                    # normalization: psum row 64 is the softmax denominator;
                    # broadcast to 64 partitions via stream_shuffle, then
                    # multiply by its (fast approx) reciprocal
                    rsb = rsbp.tile([64, HT], F32, tag="rsb")
                    nc.vector.stream_shuffle(
                        rsb[0:32, :], pO[64:96, :], mask=[0] * 32
                    )
                    nc.vector.stream_shuffle(
                        rsb[32:64, :], pO[64:96, :], mask=[0] * 32
                    )
                    scr = rsbp.tile([64, HT], F32, tag="scr")
                    nc.vector.reciprocal_approx_accurate(
                        out=rsb[:, :], in_=rsb[:, :], scratch=scr[:, :]
                    )
                    dv = rsbp.tile([64, HT], F32, tag="dv")
                    nc.vector.tensor_tensor(
                        out=dv[:, :], in0=pO[0:64, :], in1=rsb[:, :], op=ALU.mult
                    )
                    if h % 2 == 0:
                        nc.vector.tensor_copy(ONpk[0:64, eb, q0 : q0 + HT], dv[:, :])
                    else:
                        tmp = oddp.tile([64, HT], BF16, tag="odd")
                        nc.vector.tensor_copy(tmp[:, :], dv[:, :])
                        # partition shift 0-63 -> 64-127 via SBUF-to-SBUF DMA
                        nc.sync.dma_start(
                            ONpk[64:128, eb, q0 : q0 + HT], tmp[:, :]
                        )

            # ---- output projection: out[t, d] = ON[e, t].T @ wo[e, d] ----
            for tn in range(TC):
                for dn in range(2):
                    po = psp.tile([128, 512], F32, tag="ps")
                    for eb in range(EB):
                        nc.tensor.matmul(
                            po[:],
                            ONpk[:, eb, 128 * tn : 128 * tn + 128],
                            wo_sb[:, eb, 512 * dn : 512 * dn + 512],
                            start=(eb == 0),
                            stop=(eb == EB - 1),
                        )
                    ob = outsp.tile([128, 512], F32, tag="ob")
                    if (tn + dn) % 2 == 0:
                        nc.vector.tensor_copy(ob[:], po[:])
                    else:
                        nc.scalar.copy(ob[:], po[:])
                    nc.sync.dma_start(
                        outp[128 * tn : 128 * tn + 128, 512 * dn : 512 * dn + 512],
                        ob[:],
                    )
    nc.compile()
    return nc


_CACHE = {}
LAST_RESULTS = None


def get_nc():
    if "nc" not in _CACHE:
        _CACHE["nc"] = build_nc()
    return _CACHE["nc"]


def make_in_maps(q, k, v, wq, wk, wv, wo):
    q, k, v, wq, wk, wv, wo = (
        np.asarray(a, dtype=np.float32) for a in (q, k, v, wq, wk, wv, wo)
    )
    scale = 1.0 / math.sqrt(DH)
    xT = [
        (
            np.ascontiguousarray(q[b].T).astype(BF16NP),
            np.ascontiguousarray(k[b].T).astype(BF16NP),
            np.ascontiguousarray(v[b].T).astype(BF16NP),
        )
        for b in range(B)
    ]
    in_maps = []
    for c in range(NCORES):
        b, g = divmod(c, G)
        gs = slice(E * g, E * (g + 1))
        in_maps.append(
            {
                "xqT": xT[b][0],
                "xkT": xT[b][1],
                "xvT": xT[b][2],
                "wqT": np.ascontiguousarray((wq[gs] * scale).T).astype(BF16NP),
                "wkT": np.ascontiguousarray(wk[gs].T).astype(BF16NP),
                "wvT": np.ascontiguousarray(wv[gs].T).astype(BF16NP),
                "woT": np.ascontiguousarray(wo[:, gs].T).astype(BF16NP),
            }
        )
    return in_maps


def kernel(q, k, v, wq, wk, wv, wo):
    global LAST_RESULTS
    nc = get_nc()
    in_maps = make_in_maps(q, k, v, wq, wk, wv, wo)
    res = run_bass_kernel_spmd(nc, in_maps, core_ids=list(range(NCORES)))
    LAST_RESULTS = res
    out = np.zeros((B, T, D), dtype=np.float32)
    for c in range(NCORES):
        out[c // G] += res.results[c]["outp"]
    return out


# revision 22
# speedup vs baseline: 1.1974x; 1.1974x over previous
"""Multi-head causal attention (B=2, T=2048, D=1024, H=16) on 8 trn2 cores.

Sharding: core c = (batch b, head-group g) with b = c//4, g = c%4.
Each core computes Q/K/V projections for its 4 heads (256 features),
causal attention, and its partial output projection; the host sums the
4 per-batch partials (the w_o all-reduce) and stacks batches.

All device matmuls run as float32r (full PE rate, near-fp32 accuracy).
Host pre-transposes x and weight slices so every matmul operand is
contraction-major; 1/sqrt(dh) is folded into wq on the host.
"""

import math

import ml_dtypes
import numpy as np

BF16NP = ml_dtypes.bfloat16

import concourse.bass as bass
from concourse import bacc
import concourse.mybir as mybir
import concourse.tile as tile
from concourse.bass_utils import run_bass_kernel_spmd

F32 = mybir.dt.float32
F32R = mybir.dt.float32r
AF = mybir.ActivationFunctionType
ALU = mybir.AluOpType
BF16 = mybir.dt.bfloat16

B, T, D, H = 2, 2048, 1024, 16
NCORES = 8
G = 4             # head groups (tensor parallel); cores = B * G
HPG = H // G      # 4 heads per core
DH = D // H       # 64 head dim
E = D // G        # 256 features per core
EB = E // 128     # 2 e-blocks of 128
KD = D // 128     # 8 contraction chunks for projections
TN = T // 512     # 4 512-wide t stripes
TC = T // 128     # 16 128-wide k/t chunks
HT = T // 2       # 1024-wide attention q-half


def build_nc():
    nc = bacc.Bacc(None)
    xqT = nc.declare_dram_parameter("xqT", [D, T], BF16, isOutput=False)
    xkT = nc.declare_dram_parameter("xkT", [D, T], BF16, isOutput=False)
    xvT = nc.declare_dram_parameter("xvT", [D, T], BF16, isOutput=False)
    wqT = nc.declare_dram_parameter("wqT", [D, E], BF16, isOutput=False)
    wkT = nc.declare_dram_parameter("wkT", [D, E], BF16, isOutput=False)
    wvT = nc.declare_dram_parameter("wvT", [D, E], BF16, isOutput=False)
    woT = nc.declare_dram_parameter("woT", [E, D], BF16, isOutput=False)
    outp = nc.declare_dram_parameter("outp", [T, D], F32, isOutput=True)

    with tile.TileContext(nc) as tc:
        with (
            tc.tile_pool(name="persist", bufs=1) as persist,
            tc.tile_pool(name="xs", bufs=16) as xs,
            tc.tile_pool(name="pt", bufs=3) as ptp,
            tc.tile_pool(name="ptd", bufs=2) as ptdp,
            tc.tile_pool(name="rsb", bufs=2) as rsbp,
            tc.tile_pool(name="odd", bufs=2) as oddp,
            tc.tile_pool(name="outs", bufs=4) as outsp,
            tc.tile_pool(name="ps", bufs=2, space="PSUM") as psp,
            tc.tile_pool(name="po", bufs=2, space="PSUM") as pop,
        ):
            wq_sb = persist.tile([128, KD, E], BF16, tag="wq")
            wk_sb = persist.tile([128, KD, E], BF16, tag="wk")
            wv_sb = persist.tile([128, KD, E], BF16, tag="wv")
            wo_sb = persist.tile([128, EB, D], BF16, tag="wo")
            QT = persist.tile([128, EB, T], BF16, tag="QT")
            KT = persist.tile([128, EB, T], BF16, tag="KT")
            Vp = persist.tile([128, TC, HPG, DH + 1], BF16, tag="Vp")
            ONpk = persist.tile([128, EB, T], BF16, tag="ONpk")

            nc.sync.dma_start(wq_sb[:], wqT[:, :].rearrange("(c p) e -> p c e", p=128))
            nc.sync.dma_start(wk_sb[:], wkT[:, :].rearrange("(c p) e -> p c e", p=128))
            nc.sync.dma_start(wv_sb[:], wvT[:, :].rearrange("(c p) e -> p c e", p=128))
            nc.sync.dma_start(wo_sb[:], woT[:, :].rearrange("(c p) d -> p c d", p=128))

            # ones column of Vp: P @ [V | 1] makes the softmax denominator
            # fall out of the PV matmul as psum row 64
            onesb = persist.tile([128, 512], BF16, tag="onesb")
            nc.vector.memset(onesb[:], 1.0)
            nc.vector.tensor_copy(
                Vp[:, :, :, DH : DH + 1],
                onesb[:, None, None, 0:1].broadcast_to([128, TC, HPG, 1]),
            )


            def load_x_stripe(xdram, n):
                tiles = []
                xr = xdram[:, :].rearrange("(c p) t -> p c t", p=128)
                for kd in range(KD):
                    t = xs.tile([128, 512], BF16, tag="x")
                    nc.sync.dma_start(t[:], xr[:, kd, 512 * n : 512 * n + 512])
                    tiles.append(t)
                return tiles

            # ---- Q/K projections: dest[e, t] = w[d, e].T @ x[d, t] ----
            for xdram, wsb, dest in ((xqT, wq_sb, QT), (xkT, wk_sb, KT)):
                for n in range(TN):
                    xc = load_x_stripe(xdram, n)
                    for eb in range(EB):
                        acc = psp.tile([128, 512], F32, tag="ps")
                        for kd in range(KD):
                            nc.tensor.matmul(
                                acc[:],
                                wsb[:, kd, 128 * eb : 128 * eb + 128],
                                xc[kd][:],
                                start=(kd == 0),
                                stop=(kd == KD - 1),
                            )
                        nc.vector.tensor_copy(
                            dest[:, eb, 512 * n : 512 * n + 512], acc[:]
                        )

            # ---- V projection, natural layout: V[t, e] = x[d, t].T @ w[d, e] ----
            for n in range(TN):
                xc = load_x_stripe(xvT, n)
                for sub in range(4):
                    tcc = 4 * n + sub
                    acc = psp.tile([128, E], F32, tag="ps")
                    for kd in range(KD):
                        nc.tensor.matmul(
                            acc[:],
                            xc[kd][:, 128 * sub : 128 * sub + 128],
                            wv_sb[:, kd, :],
                            start=(kd == 0),
                            stop=(kd == KD - 1),
                        )
                    nc.vector.tensor_copy(
                        Vp[:, tcc, :, 0:DH],
                        acc[:].rearrange("p (h d) -> p h d", h=HPG),
                    )

            # ---- attention per (head, q-half); pO double-buffers so the
            # next half's PV accumulation overlaps this half's softmax
            # normalization ----
            for h in range(HPG):
                eb, r0 = h // 2, 64 * (h % 2)
                for half in range(2):
                    q0 = HT * half
                    pO = pop.tile([128, HT], F32, tag="po")
                    # rows 65-95 feed stream_shuffle; only row 64 is real
                    nc.vector.memset(pO[64:96, :], 0.0)
                    kc_hi = 8 * (half + 1)
                    for kc in range(kc_hi):
                        jlo = max(kc // 4, 2 * half)
                        pS = psp.tile([128, HT], F32, tag="ps")
                        pe_t = ptp.tile([128, HT], BF16, tag="pt")
                        for jg in range(jlo, 2 * half + 2):
                            o = 512 * jg - q0
                            nc.tensor.matmul(
                                pS[:, o : o + 512],
                                KT[r0 : r0 + 64, eb, 128 * kc : 128 * kc + 128],
                                QT[r0 : r0 + 64, eb, 512 * jg : 512 * jg + 512],
                                start=True,
                                stop=True,
                            )
                        vo = 512 * jlo - q0
                        nc.scalar.activation(pe_t[:, vo:], pS[:, vo:], AF.Exp)
                        for jg in range(jlo, 2 * half + 2):
                            o = 512 * jg - q0
                            if jg == kc // 4:
                                # causal mask on the diagonal stripe:
                                # keep qq >= kq + 128*(kc%4); gpsimd is idle
                                ptd = ptdp.tile([128, 512], BF16, tag="ptd")
                                nc.gpsimd.affine_select(
                                    out=ptd[:],
                                    in_=pe_t[:, o : o + 512],
                                    pattern=[[1, 512]],
                                    compare_op=ALU.is_ge,
                                    fill=0.0,
                                    base=-(128 * (kc % 4)),
                                    channel_multiplier=-1,
                                )
                                src = ptd[:]
                            else:
                                src = pe_t[:, o : o + 512]
                            nc.tensor.matmul(
                                pO[0:65, o : o + 512],
                                Vp[:, kc, h, :],
                                src,
                                start=(kc == 0),
                                stop=(kc == 4 * jg + 3),
                            )
                    # normalization: psum row 64 is the softmax denominator;
                    # broadcast to 64 partitions via stream_shuffle, then
                    # multiply by its (fast approx) reciprocal
                    rsb = rsbp.tile([64, HT], F32, tag="rsb")
                    nc.vector.stream_shuffle(
                        rsb[0:32, :], pO[64:96, :], mask=[0] * 32
                    )
                    nc.vector.stream_shuffle(
                        rsb[32:64, :], pO[64:96, :], mask=[0] * 32
                    )
                    scr = rsbp.tile([64, HT], F32, tag="scr")
                    nc.vector.reciprocal_approx_accurate(
                        out=rsb[:, :], in_=rsb[:, :], scratch=scr[:, :]
                    )
                    dv = rsbp.tile([64, HT], F32, tag="dv")
                    nc.vector.tensor_tensor(
                        out=dv[:, :], in0=pO[0:64, :], in1=rsb[:, :], op=ALU.mult
                    )
                    if h % 2 == 0:
                        nc.vector.tensor_copy(ONpk[0:64, eb, q0 : q0 + HT], dv[:, :])
                    else:
                        tmp = oddp.tile([64, HT], BF16, tag="odd")
                        nc.vector.tensor_copy(tmp[:, :], dv[:, :])
                        # partition shift 0-63 -> 64-127 via SBUF-to-SBUF DMA
                        nc.sync.dma_start(
                            ONpk[64:128, eb, q0 : q0 + HT], tmp[:, :]
                        )

            # ---- output projection: out[t, d] = ON[e, t].T @ wo[e, d] ----
            for tn in range(TC):
                for dn in range(2):
                    po = psp.tile([128, 512], F32, tag="ps")
                    for eb in range(EB):
                        nc.tensor.matmul(
                            po[:],
                            ONpk[:, eb, 128 * tn : 128 * tn + 128],
                            wo_sb[:, eb, 512 * dn : 512 * dn + 512],
                            start=(eb == 0),
                            stop=(eb == EB - 1),
                        )
                    ob = outsp.tile([128, 512], F32, tag="ob")
                    if (tn + dn) % 2 == 0:
                        nc.vector.tensor_copy(ob[:], po[:])
                    else:
                        nc.scalar.copy(ob[:], po[:])
                    nc.sync.dma_start(
                        outp[128 * tn : 128 * tn + 128, 512 * dn : 512 * dn + 512],
                        ob[:],
                    )
    nc.compile()
    return nc


_CACHE = {}
LAST_RESULTS = None


def get_nc():
    if "nc" not in _CACHE:
        _CACHE["nc"] = build_nc()
    return _CACHE["nc"]


def make_in_maps(q, k, v, wq, wk, wv, wo):
    q, k, v, wq, wk, wv, wo = (
        np.asarray(a, dtype=np.float32) for a in (q, k, v, wq, wk, wv, wo)
    )
    scale = 1.0 / math.sqrt(DH)
    xT = [
        (
            np.ascontiguousarray(q[b].T).astype(BF16NP),
            np.ascontiguousarray(k[b].T).astype(BF16NP),
            np.ascontiguousarray(v[b].T).astype(BF16NP),
        )
        for b in range(B)
    ]
    in_maps = []
    for c in range(NCORES):
        b, g = divmod(c, G)
        gs = slice(E * g, E * (g + 1))
        in_maps.append(
            {
                "xqT": xT[b][0],
                "xkT": xT[b][1],
                "xvT": xT[b][2],
                "wqT": np.ascontiguousarray((wq[gs] * scale).T).astype(BF16NP),
                "wkT": np.ascontiguousarray(wk[gs].T).astype(BF16NP),
                "wvT": np.ascontiguousarray(wv[gs].T).astype(BF16NP),
                "woT": np.ascontiguousarray(wo[:, gs].T).astype(BF16NP),
            }
        )
    return in_maps


def kernel(q, k, v, wq, wk, wv, wo):
    global LAST_RESULTS
    nc = get_nc()
    in_maps = make_in_maps(q, k, v, wq, wk, wv, wo)
    res = run_bass_kernel_spmd(nc, in_maps, core_ids=list(range(NCORES)))
    LAST_RESULTS = res
    out = np.zeros((B, T, D), dtype=np.float32)
    for c in range(NCORES):
        out[c // G] += res.results[c]["outp"]
    return out


# revision 23
# speedup vs baseline: 1.2151x; 1.0149x over previous
"""Multi-head causal attention (B=2, T=2048, D=1024, H=16) on 8 trn2 cores.

Sharding: core c = (batch b, head-group g) with b = c//4, g = c%4.
Each core computes Q/K/V projections for its 4 heads (256 features),
causal attention, and its partial output projection; the host sums the
4 per-batch partials (the w_o all-reduce) and stacks batches.

All device matmuls run as float32r (full PE rate, near-fp32 accuracy).
Host pre-transposes x and weight slices so every matmul operand is
contraction-major; 1/sqrt(dh) is folded into wq on the host.
"""

import math

import ml_dtypes
import numpy as np

BF16NP = ml_dtypes.bfloat16

import concourse.bass as bass
from concourse import bacc
import concourse.mybir as mybir
import concourse.tile as tile
from concourse.bass_utils import run_bass_kernel_spmd

F32 = mybir.dt.float32
F32R = mybir.dt.float32r
AF = mybir.ActivationFunctionType
ALU = mybir.AluOpType
BF16 = mybir.dt.bfloat16

B, T, D, H = 2, 2048, 1024, 16
NCORES = 8
G = 4             # head groups (tensor parallel); cores = B * G
HPG = H // G      # 4 heads per core
DH = D // H       # 64 head dim
E = D // G        # 256 features per core
EB = E // 128     # 2 e-blocks of 128
KD = D // 128     # 8 contraction chunks for projections
TN = T // 512     # 4 512-wide t stripes
TC = T // 128     # 16 128-wide k/t chunks
HT = T // 2       # 1024-wide attention q-half


def build_nc():
    nc = bacc.Bacc(None)
    xqT = nc.declare_dram_parameter("xqT", [D, T], BF16, isOutput=False)
    xkT = nc.declare_dram_parameter("xkT", [D, T], BF16, isOutput=False)
    xvT = nc.declare_dram_parameter("xvT", [D, T], BF16, isOutput=False)
    wqT = nc.declare_dram_parameter("wqT", [D, E], BF16, isOutput=False)
    wkT = nc.declare_dram_parameter("wkT", [D, E], BF16, isOutput=False)
    wvT = nc.declare_dram_parameter("wvT", [D, E], BF16, isOutput=False)
    woT = nc.declare_dram_parameter("woT", [E, D], BF16, isOutput=False)
    outp = nc.declare_dram_parameter("outp", [T, D], F32, isOutput=True)

    with tile.TileContext(nc) as tc:
        with (
            tc.tile_pool(name="persist", bufs=1) as persist,
            tc.tile_pool(name="xs", bufs=16) as xs,
            tc.tile_pool(name="pt", bufs=4) as ptp,
            tc.tile_pool(name="rsb", bufs=2) as rsbp,
            tc.tile_pool(name="odd", bufs=2) as oddp,
            tc.tile_pool(name="outs", bufs=4) as outsp,
            tc.tile_pool(name="ps", bufs=2, space="PSUM") as psp,
            tc.tile_pool(name="po", bufs=2, space="PSUM") as pop,
        ):
            wq_sb = persist.tile([128, KD, E], BF16, tag="wq")
            wk_sb = persist.tile([128, KD, E], BF16, tag="wk")
            wv_sb = persist.tile([128, KD, E], BF16, tag="wv")
            wo_sb = persist.tile([128, EB, D], BF16, tag="wo")
            QT = persist.tile([128, EB, T], BF16, tag="QT")
            KT = persist.tile([128, EB, T], BF16, tag="KT")
            Vp = persist.tile([128, TC, HPG, DH + 1], BF16, tag="Vp")
            ONpk = persist.tile([128, EB, T], BF16, tag="ONpk")

            nc.sync.dma_start(wq_sb[:], wqT[:, :].rearrange("(c p) e -> p c e", p=128))
            nc.sync.dma_start(wk_sb[:], wkT[:, :].rearrange("(c p) e -> p c e", p=128))
            nc.sync.dma_start(wv_sb[:], wvT[:, :].rearrange("(c p) e -> p c e", p=128))
            nc.sync.dma_start(wo_sb[:], woT[:, :].rearrange("(c p) d -> p c d", p=128))

            # ones column of Vp: P @ [V | 1] makes the softmax denominator
            # fall out of the PV matmul as psum row 64
            onesb = persist.tile([128, 512], BF16, tag="onesb")
            nc.vector.memset(onesb[:], 1.0)
            nc.vector.tensor_copy(
                Vp[:, :, :, DH : DH + 1],
                onesb[:, None, None, 0:1].broadcast_to([128, TC, HPG, 1]),
            )


            def load_x_stripe(xdram, n):
                tiles = []
                xr = xdram[:, :].rearrange("(c p) t -> p c t", p=128)
                for kd in range(KD):
                    t = xs.tile([128, 512], BF16, tag="x")
                    nc.sync.dma_start(t[:], xr[:, kd, 512 * n : 512 * n + 512])
                    tiles.append(t)
                return tiles

            # ---- Q/K projections: dest[e, t] = w[d, e].T @ x[d, t] ----
            for xdram, wsb, dest in ((xqT, wq_sb, QT), (xkT, wk_sb, KT)):
                for n in range(TN):
                    xc = load_x_stripe(xdram, n)
                    for eb in range(EB):
                        acc = psp.tile([128, 512], F32, tag="ps")
                        for kd in range(KD):
                            nc.tensor.matmul(
                                acc[:],
                                wsb[:, kd, 128 * eb : 128 * eb + 128],
                                xc[kd][:],
                                start=(kd == 0),
                                stop=(kd == KD - 1),
                            )
                        nc.vector.tensor_copy(
                            dest[:, eb, 512 * n : 512 * n + 512], acc[:]
                        )

            # ---- V projection, natural layout: V[t, e] = x[d, t].T @ w[d, e] ----
            for n in range(TN):
                xc = load_x_stripe(xvT, n)
                for sub in range(4):
                    tcc = 4 * n + sub
                    acc = psp.tile([128, E], F32, tag="ps")
                    for kd in range(KD):
                        nc.tensor.matmul(
                            acc[:],
                            xc[kd][:, 128 * sub : 128 * sub + 128],
                            wv_sb[:, kd, :],
                            start=(kd == 0),
                            stop=(kd == KD - 1),
                        )
                    nc.vector.tensor_copy(
                        Vp[:, tcc, :, 0:DH],
                        acc[:].rearrange("p (h d) -> p h d", h=HPG),
                    )

            # ---- attention per (head, q-half); pO double-buffers so the
            # next half's PV accumulation overlaps this half's softmax
            # normalization ----
            for h in range(HPG):
                eb, r0 = h // 2, 64 * (h % 2)
                for half in range(2):
                    q0 = HT * half
                    pO = pop.tile([128, HT], F32, tag="po")
                    # rows 65-95 feed stream_shuffle; only row 64 is real
                    nc.vector.memset(pO[64:96, :], 0.0)
                    kc_hi = 8 * (half + 1)
                    for kc in range(kc_hi):
                        jlo = max(kc // 4, 2 * half)
                        pS = psp.tile([128, HT], F32, tag="ps")
                        pe_t = ptp.tile([128, HT], BF16, tag="pt")
                        for jg in range(jlo, 2 * half + 2):
                            o = 512 * jg - q0
                            nc.tensor.matmul(
                                pS[:, o : o + 512],
                                KT[r0 : r0 + 64, eb, 128 * kc : 128 * kc + 128],
                                QT[r0 : r0 + 64, eb, 512 * jg : 512 * jg + 512],
                                start=True,
                                stop=True,
                            )
                        vo = 512 * jlo - q0
                        nc.scalar.activation(pe_t[:, vo:], pS[:, vo:], AF.Exp)
                        for jg in range(jlo, 2 * half + 2):
                            o = 512 * jg - q0
                            if jg == kc // 4:
                                # causal mask in place on the diagonal
                                # stripe: keep qq >= kq + 128*(kc%4)
                                nc.gpsimd.affine_select(
                                    out=pe_t[:, o : o + 512],
                                    in_=pe_t[:, o : o + 512],
                                    pattern=[[1, 512]],
                                    compare_op=ALU.is_ge,
                                    fill=0.0,
                                    base=-(128 * (kc % 4)),
                                    channel_multiplier=-1,
                                )
                            src = pe_t[:, o : o + 512]
                            nc.tensor.matmul(
                                pO[0:65, o : o + 512],
                                Vp[:, kc, h, :],
                                src,
                                start=(kc == 0),
                                stop=(kc == 4 * jg + 3),
                            )
                    # normalization: psum row 64 is the softmax denominator;
                    # broadcast to 64 partitions via stream_shuffle, then
                    # multiply by its (fast approx) reciprocal
                    rsb = rsbp.tile([64, HT], F32, tag="rsb")
                    nc.vector.stream_shuffle(
                        rsb[0:32, :], pO[64:96, :], mask=[0] * 32
                    )
                    nc.vector.stream_shuffle(
                        rsb[32:64, :], pO[64:96, :], mask=[0] * 32
                    )
                    scr = rsbp.tile([64, HT], F32, tag="scr")
                    nc.vector.reciprocal_approx_accurate(
                        out=rsb[:, :], in_=rsb[:, :], scratch=scr[:, :]
                    )
                    dv = rsbp.tile([64, HT], F32, tag="dv")
                    nc.vector.tensor_tensor(
                        out=dv[:, :], in0=pO[0:64, :], in1=rsb[:, :], op=ALU.mult
                    )
                    if h % 2 == 0:
                        nc.vector.tensor_copy(ONpk[0:64, eb, q0 : q0 + HT], dv[:, :])
                    else:
                        tmp = oddp.tile([64, HT], BF16, tag="odd")
                        nc.vector.tensor_copy(tmp[:, :], dv[:, :])
                        # partition shift 0-63 -> 64-127 via SBUF-to-SBUF DMA
                        nc.sync.dma_start(
                            ONpk[64:128, eb, q0 : q0 + HT], tmp[:, :]
                        )

            # ---- output projection: out[t, d] = ON[e, t].T @ wo[e, d] ----
            for tn in range(TC):
                for dn in range(2):
                    po = psp.tile([128, 512], F32, tag="ps")
                    for eb in range(EB):
                        nc.tensor.matmul(
                            po[:],
                            ONpk[:, eb, 128 * tn : 128 * tn + 128],
                            wo_sb[:, eb, 512 * dn : 512 * dn + 512],
                            start=(eb == 0),
                            stop=(eb == EB - 1),
                        )
                    ob = outsp.tile([128, 512], F32, tag="ob")
                    if (tn + dn) % 2 == 0:
                        nc.vector.tensor_copy(ob[:], po[:])
                    else:
                        nc.scalar.copy(ob[:], po[:])
                    nc.sync.dma_start(
                        outp[128 * tn : 128 * tn + 128, 512 * dn : 512 * dn + 512],
                        ob[:],
                    )
    nc.compile()
    return nc


_CACHE = {}
LAST_RESULTS = None


def get_nc():
    if "nc" not in _CACHE:
        _CACHE["nc"] = build_nc()
    return _CACHE["nc"]


def make_in_maps(q, k, v, wq, wk, wv, wo):
    q, k, v, wq, wk, wv, wo = (
        np.asarray(a, dtype=np.float32) for a in (q, k, v, wq, wk, wv, wo)
    )
    scale = 1.0 / math.sqrt(DH)
    xT = [
        (
            np.ascontiguousarray(q[b].T).astype(BF16NP),
            np.ascontiguousarray(k[b].T).astype(BF16NP),
            np.ascontiguousarray(v[b].T).astype(BF16NP),
        )
        for b in range(B)
    ]
    in_maps = []
    for c in range(NCORES):
        b, g = divmod(c, G)
        gs = slice(E * g, E * (g + 1))
        in_maps.append(
            {
                "xqT": xT[b][0],
                "xkT": xT[b][1],
                "xvT": xT[b][2],
                "wqT": np.ascontiguousarray((wq[gs] * scale).T).astype(BF16NP),
                "wkT": np.ascontiguousarray(wk[gs].T).astype(BF16NP),
                "wvT": np.ascontiguousarray(wv[gs].T).astype(BF16NP),
                "woT": np.ascontiguousarray(wo[:, gs].T).astype(BF16NP),
            }
        )
    return in_maps


def kernel(q, k, v, wq, wk, wv, wo):
    global LAST_RESULTS
    nc = get_nc()
    in_maps = make_in_maps(q, k, v, wq, wk, wv, wo)
    res = run_bass_kernel_spmd(nc, in_maps, core_ids=list(range(NCORES)))
    LAST_RESULTS = res
    out = np.zeros((B, T, D), dtype=np.float32)
    for c in range(NCORES):
        out[c // G] += res.results[c]["outp"]
    return out
